# revision 2
# baseline (speedup 1.0000x reference)
"""CompressedFP8Linear on 8 trn2 NeuronCores.

out[B,S,O] = x @ (weight * weight_scale).T + bias
  x:[4,32,8192] f32, weight:[8192,8192] f32 (fp8-e4m3 representable),
  weight_scale:[8192,1] f32, bias:[8192] f16.

Strategy (column-parallel, per sharding hint):
  - Shard weight rows (out_features) across 8 cores; replicate x.
  - The weight values are EXACTLY fp8-e4m3 representable (the module
    stores fp8 and dequantizes), so the host re-casts them to fp8 and
    the kernel streams 8 MiB/core instead of 32 MiB — a 4x cut in HBM
    traffic for this memory-bound problem.  Matmuls run in fp8 with
    perf_mode=DoubleRow (2 k-subtiles per instruction, 2x bf16 rate).
  - x is split hi/lo: x_hi = e4m3(x), x_lo = e5m2(x - x_hi).  Both
    passes accumulate into the same PSUM group, so the result is exact
    in the weights and ~2e-3 relative in x — far inside the 2e-2 gate.
  - Host-side marshalling (layout only): pack x and each weight shard
    k-major as [p, kt, free] with k = p*KT + kt so every SBUF
    partition's DMA reads are contiguous DRAM runs.
  - scale/bias arrive as [1, O_shard] rows and are broadcast to the 128
    token partitions on-chip (exact fp32 ones-outer-product on the PE,
    which is idle at startup).  The per-output-channel dequant scale is
    applied to the [128, O] PSUM output (64x fewer multiplies than
    dequantizing the weight), bias added on the vector engine.
  - No collectives; the host concatenates the 8 output shards.

Memory floor per core: 8 MiB weight + 2 MiB x + 0.5 MiB out.
"""

import numpy as np
import ml_dtypes

import concourse.bass as bass
import concourse.mybir as mybir
import concourse.tile as tile
from concourse.bass_utils import run_bass_kernel_spmd

B, S, IN, OUT = 4, 32, 8192, 8192
M = B * S                      # 128 tokens
NCORES = 8
OSH = OUT // NCORES            # 1024 out-features per core
KT = IN // 128                 # 64 k-tiles of 128
F32 = mybir.dt.float32
FP8H = mybir.dt.float8e4       # x_hi, weight
FP8L = mybir.dt.float8e5       # x_lo residual
DR = mybir.MatmulPerfMode.DoubleRow


def split_waits(nc, max_waits=1):
    """This walrus build encodes at most one sem-wait per instruction;
    move any excess onto NoOps injected just before (same engine queue,
    so ordering semantics are identical)."""
    n = 0
    for f in nc.m.functions:
        for bb in f.blocks:
            out = []
            for inst in bb.instructions:
                si = inst.sync_info
                waits = list(si.on_wait) if si and si.on_wait else []
                if len(waits) > max_waits:
                    extra, keep = waits[:-max_waits], waits[-max_waits:]
                    for i, w in enumerate(extra):
                        out.append(mybir.InstNoOp(
                            name=f"{inst.name}-ws{i}", engine=inst.engine,
                            ins=[], outs=[],
                            sync_info=mybir.SyncInfo(on_wait=[w], on_update=[])))
                        n += 1
                    si.on_wait = keep
                out.append(inst)
            bb.instructions = out
    return n


def build(reps=1, loop=None, slab_kt=4, w_engines=("sync", "scalar"),
          x_engine="gpsimd"):
    """One column-parallel shard: out[128, OSH] = (x_hi + x_lo) @ W * scale + bias.

    reps > 1 unrolls the whole body (including all DMA) back-to-back for
    wall-clock timing; loop=L wraps the body in a For_i hardware loop
    (timing only — amortizes the per-launch tunnel overhead).
    """
    nc = bass.Bass()
    xh_d = nc.dram_tensor("xh", [128, KT, M], FP8H, kind="ExternalInput")
    xl_d = nc.dram_tensor("xl", [128, KT, M], FP8L, kind="ExternalInput")
    wt_d = nc.dram_tensor("wt", [128, KT, OSH], FP8H, kind="ExternalInput")
    sc_d = nc.dram_tensor("scale_r", [1, OSH], F32, kind="ExternalInput")
    bi_d = nc.dram_tensor("bias_r", [1, OSH], F32, kind="ExternalInput")
    out_d = nc.dram_tensor("out", [M, OSH], F32, kind="ExternalOutput")

    wt3 = wt_d[:]                                               # [128, KT, OSH]

    # slab plan over kt in even-sized slabs (DoubleRow consumes k-pairs)
    slabs = []
    k0 = 0
    while k0 < KT - 4:
        slabs.append((k0, slab_kt))
        k0 += slab_kt
    while k0 < KT:
        slabs.append((k0, 2))
        k0 += 2

    with tile.TileContext(nc) as tc:
        with (
            tc.tile_pool(name="xp", bufs=2) as xp,
            tc.tile_pool(name="wp", bufs=4) as wp,
            tc.tile_pool(name="cp", bufs=1) as cp,
            tc.tile_pool(name="op", bufs=2) as op,
            tc.tile_pool(name="ps", bufs=2, space="PSUM") as ps,
        ):
            x_eng = getattr(nc, x_engine)
            w_engs = [getattr(nc, e) for e in w_engines]

            # broadcast scale/bias rows to all 128 partitions on-chip:
            # exact fp32 outer product with a ones column on the (still
            # idle) PE, instead of streaming 1 MiB of replicated data
            ones = cp.tile([1, M], F32)
            nc.vector.memset(ones[:], 1.0)
            sc = cp.tile([M, OSH], F32)
            bi = cp.tile([M, OSH], F32)
            for row_d, dst in ((sc_d, sc), (bi_d, bi)):
                row = cp.tile([1, OSH], F32, tag="crow")
                x_eng.dma_start(row[:], row_d[:])
                pb = ps.tile([M, OSH], F32, tag="pbcast")
                for og in range(2):
                    nc.tensor.matmul(
                        pb[:, og * 512:(og + 1) * 512],
                        ones[:, :], row[:, og * 512:(og + 1) * 512],
                        start=True, stop=True)
                nc.vector.tensor_copy(dst[:], pb[:])

            import contextlib
            lctx = tc.For_i(0, loop) if loop else contextlib.nullcontext()
            with lctx:
                for _ in range(reps):
                    # x: 2 MiB in 4 chunks so the first matmuls wait only
                    # on the first quarter
                    xh = xp.tile([128, KT, M], FP8H, tag="xh")
                    xl = xp.tile([128, KT, M], FP8L, tag="xl")
                    per = KT // 2
                    for i in range(2):
                        ksl = slice(i * per, (i + 1) * per)
                        x_eng.dma_start(xh[:, ksl, :], xh_d[:, ksl, :])
                        x_eng.dma_start(xl[:, ksl, :], xl_d[:, ksl, :])

                    acc0 = ps.tile([M, 512], F32)
                    acc1 = ps.tile([M, 512], F32)
                    accs = (acc0, acc1)
                    for t, (k0, n) in enumerate(slabs):
                        wsb = wp.tile([128, slab_kt, OSH], FP8H, tag="wsb")
                        # spread weight DMAs over rings so they pipeline
                        w_engs[t % len(w_engs)].dma_start(
                            wsb[:, :n, :], wt3[:, k0:k0 + n, :])
                        for s in range(0, n, 2):
                            k = k0 + s
                            first, last = (k == 0), (k == KT - 2)
                            for xs, st, sp in ((xh, first, False),
                                               (xl, False, last)):
                                for og in range(2):
                                    nc.tensor.matmul(
                                        accs[og][:, :],
                                        xs[:, k:k + 2, :],
                                        wsb[:, s:s + 2, og * 512:(og + 1) * 512],
                                        start=st and og < 2, stop=sp,
                                        perf_mode=DR)

                    outsb = op.tile([M, OSH], F32)
                    for og in range(2):
                        osl = outsb[:, og * 512:(og + 1) * 512]
                        nc.vector.tensor_mul(osl, accs[og][:, :],
                                             sc[:, og * 512:(og + 1) * 512])
                        nc.vector.tensor_add(osl, osl,
                                             bi[:, og * 512:(og + 1) * 512])
                        # write each half back as soon as its scale/bias done
                        x_eng.dma_start(out_d[:, og * 512:(og + 1) * 512], osl)

    split_waits(nc)
    return nc


def shard_inputs(x, weight, weight_scale, bias):
    """Host-side marshalling into per-core input maps (layout + dtype cast).

    The weight cast to fp8-e4m3 is exact: the module's stored weight is an
    fp8 round-trip, and every such value (|w| < 240) is representable in
    ml_dtypes.float8_e4m3.  x is decomposed as x_hi + x_lo with x_hi =
    e4m3(x) and x_lo = e5m2 residual.
    """
    x = np.asarray(x, dtype=np.float32)
    weight = np.asarray(weight, dtype=np.float32)
    scale = np.asarray(weight_scale, dtype=np.float32).reshape(OUT)
    bias32 = np.asarray(bias).astype(np.float32)

    # pack x k-major as [p, kt, m] with k = p*KT + kt: each partition's
    # data is one contiguous DRAM run
    xt = np.ascontiguousarray(x.reshape(M, IN).T).reshape(128, KT, M)
    xh = xt.astype(ml_dtypes.float8_e4m3)
    xl = (xt - xh.astype(np.float32)).astype(ml_dtypes.float8_e5m2)

    in_maps = []
    for c in range(NCORES):
        sl = slice(c * OSH, (c + 1) * OSH)
        wt = np.ascontiguousarray(weight[sl, :].T)              # [IN, OSH]
        w8 = wt.reshape(128, KT, OSH).astype(ml_dtypes.float8_e4m3)
        in_maps.append({
            "xh": xh, "xl": xl, "wt": w8,
            "scale_r": np.ascontiguousarray(scale[sl][None, :]),
            "bias_r": np.ascontiguousarray(bias32[sl][None, :]),
        })
    return in_maps


def kernel(x, weight, weight_scale, bias):
    nc = build(reps=1)
    in_maps = shard_inputs(x, weight, weight_scale, bias)
    res = run_bass_kernel_spmd(nc, in_maps, core_ids=list(range(NCORES)))
    out = np.concatenate([res.results[c]["out"] for c in range(NCORES)], axis=1)
    return out.reshape(B, S, OUT)


# revision 7
# speedup vs baseline: 3.9467x; 3.9467x over previous
"""CompressedFP8Linear on 8 trn2 NeuronCores.

out[B,S,O] = x @ (weight * weight_scale).T + bias
  x:[4,32,8192] f32, weight:[8192,8192] f32 (fp8-e4m3 representable),
  weight_scale:[8192,1] f32, bias:[8192] f16.

Strategy (column-parallel, per sharding hint):
  - Shard weight rows (out_features) across 8 cores; replicate x.
  - The weight values are EXACTLY fp8-e4m3 representable (the module
    stores fp8 and dequantizes), so the host re-casts them to fp8 and
    the kernel streams 8 MiB/core instead of 32 MiB — a 4x cut in HBM
    traffic for this memory-bound problem.  Matmuls run in fp8 with
    perf_mode=DoubleRow (2 k-subtiles per instruction, 2x bf16 rate).
  - x is split hi/lo: x_hi = e4m3(x), x_lo = e5m2(x - x_hi).  Both
    passes accumulate into the same PSUM group, so the result is exact
    in the weights and ~2e-3 relative in x — far inside the 2e-2 gate.
  - Host-side marshalling (layout only): pack x and each weight shard
    k-major as [p, kt, free] with k = p*KT + kt so every SBUF
    partition's DMA reads are contiguous DRAM runs.
  - scale/bias arrive as [1, O_shard] rows and are broadcast to the 128
    token partitions on-chip (exact fp32 ones-outer-product on the PE,
    which is idle at startup).  The per-output-channel dequant scale is
    applied to the [128, O] PSUM output (64x fewer multiplies than
    dequantizing the weight), bias added on the vector engine.
  - No collectives; the host concatenates the 8 output shards.

Memory floor per core: 8 MiB weight + 2 MiB x + 0.5 MiB out.
"""

import numpy as np
import ml_dtypes

import concourse.bass as bass
import concourse.mybir as mybir
import concourse.tile as tile
from concourse.bass_utils import run_bass_kernel_spmd

B, S, IN, OUT = 4, 32, 8192, 8192
M = B * S                      # 128 tokens
NCORES = 8
OSH = OUT // NCORES            # 1024 out-features per core
KT = IN // 128                 # 64 k-tiles of 128
F32 = mybir.dt.float32
FP8H = mybir.dt.float8e4       # x_hi, weight
FP8L = mybir.dt.float8e5       # x_lo residual
DR = mybir.MatmulPerfMode.DoubleRow


def split_waits(nc, max_waits=1):
    """This walrus build encodes at most one sem-wait per instruction;
    move any excess onto NoOps injected just before (same engine queue,
    so ordering semantics are identical)."""
    n = 0
    for f in nc.m.functions:
        for bb in f.blocks:
            out = []
            for inst in bb.instructions:
                si = inst.sync_info
                waits = list(si.on_wait) if si and si.on_wait else []
                if len(waits) > max_waits:
                    extra, keep = waits[:-max_waits], waits[-max_waits:]
                    for i, w in enumerate(extra):
                        out.append(mybir.InstNoOp(
                            name=f"{inst.name}-ws{i}", engine=inst.engine,
                            ins=[], outs=[],
                            sync_info=mybir.SyncInfo(on_wait=[w], on_update=[])))
                        n += 1
                    si.on_wait = keep
                out.append(inst)
            bb.instructions = out
    return n


def build(reps=1, loop=None, slab_kt=8, w_engines=("sync", "scalar"),
          x_engine="gpsimd", out_bf16=True, wp_bufs=4):
    """One column-parallel shard: out[128, OSH] = (x_hi + x_lo) @ W * scale + bias.

    reps > 1 unrolls the whole body (including all DMA) back-to-back for
    wall-clock timing; loop=L wraps the body in a For_i hardware loop
    (timing only — amortizes the per-launch tunnel overhead).
    """
    nc = bass.Bass()
    xh_d = nc.dram_tensor("xh", [128, KT, M], FP8H, kind="ExternalInput")
    xl_d = nc.dram_tensor("xl", [128, KT, M], FP8L, kind="ExternalInput")
    wt_d = nc.dram_tensor("wt", [128, KT, OSH], FP8H, kind="ExternalInput")
    sc_d = nc.dram_tensor("scale_r", [1, OSH], F32, kind="ExternalInput")
    bi_d = nc.dram_tensor("bias_r", [1, OSH], F32, kind="ExternalInput")
    OUT_DT = mybir.dt.bfloat16 if out_bf16 else F32
    out_d = nc.dram_tensor("out", [M, OSH], OUT_DT, kind="ExternalOutput")

    wt3 = wt_d[:]                                               # [128, KT, OSH]

    # slab plan over kt in even-sized slabs (DoubleRow consumes k-pairs)
    slabs = []
    k0 = 0
    while k0 < KT - 4:
        slabs.append((k0, slab_kt))
        k0 += slab_kt
    while k0 < KT:
        slabs.append((k0, 2))
        k0 += 2

    with tile.TileContext(nc) as tc:
        with (
            tc.tile_pool(name="xp", bufs=2) as xp,
            tc.tile_pool(name="wp", bufs=wp_bufs) as wp,
            tc.tile_pool(name="cp", bufs=1) as cp,
            tc.tile_pool(name="op", bufs=2) as op,
            tc.tile_pool(name="ps", bufs=2, space="PSUM") as ps,
        ):
            x_eng = getattr(nc, x_engine)
            w_engs = [getattr(nc, e) for e in w_engines]

            # broadcast scale/bias rows to all 128 partitions on-chip:
            # exact fp32 outer product with a ones column on the (still
            # idle) PE, instead of streaming 1 MiB of replicated data
            ones = cp.tile([1, M], F32)
            nc.vector.memset(ones[:], 1.0)
            sc = cp.tile([M, OSH], F32)
            bi = cp.tile([M, OSH], F32)
            for row_d, dst in ((sc_d, sc), (bi_d, bi)):
                row = cp.tile([1, OSH], F32, tag="crow")
                x_eng.dma_start(row[:], row_d[:])
                pb = ps.tile([M, OSH], F32, tag="pbcast")
                for og in range(2):
                    nc.tensor.matmul(
                        pb[:, og * 512:(og + 1) * 512],
                        ones[:, :], row[:, og * 512:(og + 1) * 512],
                        start=True, stop=True)
                nc.vector.tensor_copy(dst[:], pb[:])

            import contextlib
            lctx = tc.For_i(0, loop) if loop else contextlib.nullcontext()
            with lctx:
                for _ in range(reps):
                    # x: 2 MiB in 4 chunks so the first matmuls wait only
                    # on the first quarter
                    xh = xp.tile([128, KT, M], FP8H, tag="xh")
                    xl = xp.tile([128, KT, M], FP8L, tag="xl")
                    per = KT // 2
                    for i in range(2):
                        ksl = slice(i * per, (i + 1) * per)
                        x_eng.dma_start(xh[:, ksl, :], xh_d[:, ksl, :])
                        x_eng.dma_start(xl[:, ksl, :], xl_d[:, ksl, :])

                    acc0 = ps.tile([M, 512], F32)
                    acc1 = ps.tile([M, 512], F32)
                    accs = (acc0, acc1)
                    for t, (k0, n) in enumerate(slabs):
                        wsb = wp.tile([128, slab_kt, OSH], FP8H, tag="wsb")
                        # spread weight DMAs over rings so they pipeline
                        w_engs[t % len(w_engs)].dma_start(
                            wsb[:, :n, :], wt3[:, k0:k0 + n, :])
                        for s in range(0, n, 2):
                            k = k0 + s
                            first, last = (k == 0), (k == KT - 2)
                            for xs, st, sp in ((xh, first, False),
                                               (xl, False, last)):
                                for og in range(2):
                                    nc.tensor.matmul(
                                        accs[og][:, :],
                                        xs[:, k:k + 2, :],
                                        wsb[:, s:s + 2, og * 512:(og + 1) * 512],
                                        start=st and og < 2, stop=sp,
                                        perf_mode=DR)

                    outsb = op.tile([M, OSH], OUT_DT)
                    tmp = op.tile([M, 512], F32, tag="otmp")
                    for og in range(2):
                        osl = outsb[:, og * 512:(og + 1) * 512]
                        # mul in f32, add converts on write to OUT_DT
                        nc.vector.tensor_mul(tmp[:], accs[og][:, :],
                                             sc[:, og * 512:(og + 1) * 512])
                        nc.vector.tensor_add(osl, tmp[:],
                                             bi[:, og * 512:(og + 1) * 512])
                        # write each half back as soon as its scale/bias done
                        x_eng.dma_start(out_d[:, og * 512:(og + 1) * 512], osl)

    split_waits(nc)
    return nc


def shard_inputs(x, weight, weight_scale, bias):
    """Host-side marshalling into per-core input maps (layout + dtype cast).

    The weight cast to fp8-e4m3 is exact: the module's stored weight is an
    fp8 round-trip, and every such value (|w| < 240) is representable in
    ml_dtypes.float8_e4m3.  x is decomposed as x_hi + x_lo with x_hi =
    e4m3(x) and x_lo = e5m2 residual.
    """
    x = np.asarray(x, dtype=np.float32)
    weight = np.asarray(weight, dtype=np.float32)
    scale = np.asarray(weight_scale, dtype=np.float32).reshape(OUT)
    bias32 = np.asarray(bias).astype(np.float32)

    # pack x k-major as [p, kt, m] with k = p*KT + kt: each partition's
    # data is one contiguous DRAM run
    xt = np.ascontiguousarray(x.reshape(M, IN).T).reshape(128, KT, M)
    xh = xt.astype(ml_dtypes.float8_e4m3)
    xl = (xt - xh.astype(np.float32)).astype(ml_dtypes.float8_e5m2)

    in_maps = []
    for c in range(NCORES):
        sl = slice(c * OSH, (c + 1) * OSH)
        wt = np.ascontiguousarray(weight[sl, :].T)              # [IN, OSH]
        w8 = wt.reshape(128, KT, OSH).astype(ml_dtypes.float8_e4m3)
        in_maps.append({
            "xh": xh, "xl": xl, "wt": w8,
            "scale_r": np.ascontiguousarray(scale[sl][None, :]),
            "bias_r": np.ascontiguousarray(bias32[sl][None, :]),
        })
    return in_maps


def kernel(x, weight, weight_scale, bias):
    nc = build(reps=1)
    in_maps = shard_inputs(x, weight, weight_scale, bias)
    res = run_bass_kernel_spmd(nc, in_maps, core_ids=list(range(NCORES)))
    out = np.concatenate(
        [np.asarray(res.results[c]["out"], dtype=np.float32)
         for c in range(NCORES)], axis=1)
    return out.reshape(B, S, OUT)
